# revision 4
# baseline (speedup 1.0000x reference)
"""Multi-head attention (B=2, S=2048, H=1024, 16 heads) on 8 NeuronCores.

Sharding: 2 batches x 4 head-groups (4 heads per core).  Each core gets its
batch's hidden states pre-transposed on the host ([hid, seq], so no on-device
hs transposes), computes QKV for its 4 heads, attention over its batch, and a
partial output projection.  Host sums 4 partials per batch (the all-reduce)
and adds the bias terms that commute to the end (v-bias through dense, dense
bias).

Per-core layout (matmul inputs bf16 except the QKV projection, which runs
f32r so Q/K derive from unquantized inputs; everything accumulates f32 in
PSUM):
  hsT_sb [128, 8, 2048] f32r   hidden^T; partition = hid%128, chunk = hid//128
  QTz/KTz [128, 2, 2048] bf16  pair-packed: rows 0-63 = even head d, rows
                               64-127 = odd head d; scores contract 64 rows
  Vn [128, 16, 4, 66] bf16     natural V: partition = k%128; col 64 is 1.0 so
                               the P@V matmul also emits softmax denominators
  PT [128, 4, 1024] bf16       exp(scores) ring, partition = k within chunk
  ctxTz [128, 2, 2048] bf16    unnormalized ctx^T, pair-packed
  ctxn  [128, 2, 2048] bf16    normalized ctx^T (transpose -> per-q scale ->
                               transpose back), feeds pair-accumulated dense
                               matmuls -- no epilogue combine needed.
"""

import os
import sys
import types

sys.path.insert(0, "/opt/trn_rl_repo")

import numpy as np
import ml_dtypes


def _install_ntff_shim():
    if "antenv.axon_hooks" in sys.modules:
        return
    try:
        from trn_agent_boot.trn_boot import _ntff_profile_via_ctypes
        so = "/opt/axon/libaxon_pjrt.so"
        if not os.path.exists(so):
            return
        hook = _ntff_profile_via_ctypes(so)
        mod = types.ModuleType("antenv.axon_hooks")
        mod.get_axon_ntff_profile_hook = lambda: hook
        mod.set_axon_ntff_profile_hook = lambda h: None
        sys.modules["antenv.axon_hooks"] = mod
    except Exception:
        pass


_install_ntff_shim()

import concourse.bass as bass
import concourse.mybir as mybir
import concourse.tile as tile
from concourse import bacc
from concourse.bass_utils import run_bass_kernel_spmd
from concourse.masks import make_identity

F32 = mybir.dt.float32
F32R = mybir.dt.float32r
BF16 = mybir.dt.bfloat16
EXP = mybir.ActivationFunctionType.Exp
BF = ml_dtypes.bfloat16

B, S, HID = 2, 2048, 1024
HEADS, D = 16, 64
NCORES = 8
GROUPS = 4                  # head-groups per batch
HPC = HEADS // GROUPS       # heads per core = 4
NPAIR = HPC // 2            # head pairs per core = 2
CW = HPC * D                # per-core width = 256
NHB = HID // 128            # 8
WSEQ = 512
NWIN = S // WSEQ            # 4
QW = 1024
NQW = S // QW               # 2
NKT = S // 128              # 16
RING = 4


def build_nc():
    nc = bacc.Bacc("TRN2", target_bir_lowering=False, debug=False,
                   num_devices=NCORES)

    hsT = nc.dram_tensor("hsT", [HID, S], BF16, kind="ExternalInput")
    wq = nc.dram_tensor("wq", [HID, CW], BF16, kind="ExternalInput")
    wk = nc.dram_tensor("wk", [HID, CW], BF16, kind="ExternalInput")
    wv = nc.dram_tensor("wv", [HID, CW], BF16, kind="ExternalInput")
    bq = nc.dram_tensor("bq", [CW, 1], F32, kind="ExternalInput")
    bk = nc.dram_tensor("bk", [CW, 1], F32, kind="ExternalInput")
    wd = nc.dram_tensor("wd", [CW, HID], BF16, kind="ExternalInput")
    out = nc.dram_tensor("out", [S, HID], BF16, kind="ExternalOutput")

    with tile.TileContext(nc) as tc:
        with (
            tc.tile_pool(name="persist", bufs=1) as pp,
            tc.tile_pool(name="vstg", bufs=2) as vsp,
            tc.tile_pool(name="cq", bufs=4) as cqp,
            tc.tile_pool(name="ob", bufs=6) as obp,
        ):
            identf = pp.tile([128, 128], F32)
            make_identity(nc, identf[:])
            identb = pp.tile([128, 128], BF16)
            nc.vector.tensor_copy(identb[:], identf[:])

            # ---- weight / input DMAs (weights on gpsimd queue, hsT on
            # sync queue, window-sliced so compute starts early) ----
            wq_sb = pp.tile([128, NHB, CW], BF16)
            wk_sb = pp.tile([128, NHB, CW], BF16)
            wv_sb = pp.tile([128, NHB, CW], BF16)
            wq_re = wq.ap().rearrange("(c p) m -> p c m", p=128)
            wk_re = wk.ap().rearrange("(c p) m -> p c m", p=128)
            wv_re = wv.ap().rearrange("(c p) m -> p c m", p=128)
            hsT_sb = pp.tile([128, NHB, S], BF16)
            hsT_re = hsT.ap().rearrange("(c p) s -> p c s", p=128)
            bq_sb = pp.tile([128, NPAIR, 1], F32)
            bk_sb = pp.tile([128, NPAIR, 1], F32)
            # first QKV matmul needs only wq chunk 0 and hsT[w0, hb01]:
            # split loads finely and round-robin the three DMA-capable
            # queues so multiple DMA engines run in parallel.
            nc.gpsimd.dma_start(wq_sb[:, 0:4, :], wq_re[:, 0:4, :])
            nc.sync.dma_start(hsT_sb[:, 0:2, 0:WSEQ], hsT_re[:, 0:2, 0:WSEQ])
            nc.scalar.dma_start(hsT_sb[:, 2:4, 0:WSEQ],
                                hsT_re[:, 2:4, 0:WSEQ])
            nc.sync.dma_start(hsT_sb[:, 4:6, 0:WSEQ], hsT_re[:, 4:6, 0:WSEQ])
            nc.scalar.dma_start(hsT_sb[:, 6:8, 0:WSEQ],
                                hsT_re[:, 6:8, 0:WSEQ])
            nc.gpsimd.dma_start(wq_sb[:, 4:8, :], wq_re[:, 4:8, :])
            nc.gpsimd.dma_start(
                bq_sb[:], bq.ap().rearrange("(r p) o -> p r o", p=128))
            nc.gpsimd.dma_start(wk_sb[:], wk_re[:])
            nc.gpsimd.dma_start(
                bk_sb[:], bk.ap().rearrange("(r p) o -> p r o", p=128))
            nc.gpsimd.dma_start(wv_sb[:], wv_re[:])
            wq_q = {1: (nc.sync, nc.scalar), 2: (nc.sync, nc.scalar),
                    3: (nc.gpsimd, nc.gpsimd)}
            for w in range(1, NWIN):
                wsl = slice(w * WSEQ, (w + 1) * WSEQ)
                for g, hbg in enumerate(range(0, NHB, 4)):
                    wq_q[w][g].dma_start(
                        hsT_sb[:, hbg:hbg + 4, wsl],
                        hsT_re[:, hbg:hbg + 4, wsl])
            wd_sb = pp.tile([128, NPAIR, HID], BF16)
            nc.gpsimd.dma_start(
                wd_sb[:], wd.ap().rearrange("(r p) m -> p r m", p=128))

            # ---- persistent activation tiles ----
            QTz = pp.tile([128, HPC, S], BF16)
            KTz = pp.tile([128, HPC, S], BF16)
            Vn = pp.tile([128, NKT, HPC, 66], BF16)
            PT = pp.tile([128, RING, QW], BF16)
            ctxTz = pp.tile([128, NPAIR, S], BF16)
            ctxn = pp.tile([128, NPAIR, S], BF16)
            den2 = pp.tile([128, QW], F32)
            denr = pp.tile([128, S // 128, HPC], F32)

            # zero rows 64-127 of each per-head Q/K tile (padded
            # 128-row contraction keeps the PE out of 64-row mode)
            nc.vector.memset(QTz[64:128, :, :], 0.0)
            nc.vector.memset(KTz[64:128, :, :], 0.0)

            # ones column for the denominator trick
            ones_st = pp.tile([128, NKT * HPC], F32)
            nc.vector.memset(ones_st[:], 1.0)
            nc.vector.tensor_copy(
                Vn[:, :, :, 64:65],
                ones_st[:].rearrange("p (c h) -> p c h", c=NKT)
                .rearrange("p c h -> p c h ()"))

            # ---------------- phase 1: QKV projections ----------------
            with (
                tc.tile_pool(name="pqk", bufs=1,
                             space=bass.MemorySpace.PSUM) as pqk,
                tc.tile_pool(name="pvt", bufs=2,
                             space=bass.MemorySpace.PSUM) as pvt,
            ):
                for w in range(NWIN):
                    wsl = slice(w * WSEQ, (w + 1) * WSEQ)
                    for tgt in range(3):
                        wsb = (wq_sb, wk_sb, wv_sb)[tgt]
                        for pr in range(NPAIR):
                            ps = pqk.tile([128, WSEQ], F32,
                                          tag=f"qkv{tgt}{pr}")
                            for hb in range(NHB):
                                nc.tensor.matmul(
                                    ps[:], wsb[:, hb, pr * 128:(pr + 1) * 128],
                                    hsT_sb[:, hb, wsl],
                                    start=(hb == 0), stop=(hb == NHB - 1))
                            if tgt < 2:
                                dst = (QTz, KTz)[tgt]
                                bias = (bq_sb, bk_sb)[tgt]
                                for j in range(2):
                                    nc.vector.tensor_scalar_add(
                                        dst[0:64, 2 * pr + j, wsl],
                                        ps[j * 64:(j + 1) * 64, :],
                                        bias[j * 64:(j + 1) * 64, pr, 0:1])
                            else:
                                vtw = vsp.tile([128, WSEQ], BF16)
                                nc.vector.tensor_copy(vtw[:], ps[:])
                                vps = pvt.tile([128, 256], F32, tag="vt")
                                for i in range(4):
                                    nc.tensor.transpose(
                                        vps[:, i * 64:(i + 1) * 64]
                                        .bitcast(BF16),
                                        vtw[:, i * 128:(i + 1) * 128],
                                        identb[:])
                                ch0 = (w * WSEQ) // 128
                                for i in range(4):
                                    nc.vector.tensor_copy(
                                        Vn[:, ch0 + i, 2 * pr:2 * pr + 2,
                                           0:64],
                                        vps[:, i * 64:(i + 1) * 64]
                                        .bitcast(BF16)
                                        .rearrange("p (h d) -> p h d", h=2))

            # ---------------- phase 2: attention + dense ----------------
            # ctx normalization (transpose -> per-q scale -> transpose back)
            # for window qw runs interleaved inside window qw+1's attention
            # kt loop, where the PE has slack under the ACT-bound exp pace.
            pend = []

            def emit_ctxn_unit(tpool, uqw, stl, pr, cp_eng):
                st = uqw * 8 + stl
                ssl = slice(st * 128, (st + 1) * 128)
                tp1 = tpool.tile([128, 64], F32, tag="t")
                nc.tensor.transpose(tp1[:].bitcast(BF16),
                                    ctxTz[:, pr, ssl], identb[:])
                ctxq = cqp.tile([128, 128], BF16)
                for j in range(2):
                    nc.vector.tensor_scalar_mul(
                        ctxq[:, j * 64:(j + 1) * 64],
                        tp1[:].bitcast(BF16)[:, j * 64:(j + 1) * 64],
                        denr[:, st, 2 * pr + j:2 * pr + j + 1])
                tp2 = tpool.tile([128, 64], F32, tag="t")
                nc.tensor.transpose(tp2[:].bitcast(BF16), ctxq[:], identb[:])
                if cp_eng == "act":
                    nc.scalar.copy(ctxn[:, pr, ssl], tp2[:].bitcast(BF16))
                else:
                    nc.vector.tensor_copy(ctxn[:, pr, ssl],
                                          tp2[:].bitcast(BF16))

            for qw in range(NQW):
                q0 = qw * QW
                with (
                    tc.tile_pool(name=f"pst{qw}", bufs=2,
                                 space=bass.MemorySpace.PSUM) as pst,
                    tc.tile_pool(name=f"ppv{qw}", bufs=1,
                                 space=bass.MemorySpace.PSUM) as ppv,
                    tc.tile_pool(name=f"pdx{qw}", bufs=2,
                                 space=bass.MemorySpace.PSUM) as pdx,
                ):
                    for hh in range(HPC):
                        pr, dr = hh // 2, (hh % 2) * 64
                        pva = ppv.tile([D + 1, 512], F32, tag="pva")
                        pvb = ppv.tile([D + 1, 512], F32, tag="pvb")

                        def scores(kt):
                            rg = kt % RING
                            stp = pst.tile([128, QW], F32, tag="st")
                            for qh in range(2):
                                nc.tensor.matmul(
                                    stp[:, qh * 512:(qh + 1) * 512],
                                    KTz[:, hh, kt * 128:(kt + 1) * 128],
                                    QTz[:, hh,
                                        q0 + qh * 512:q0 + (qh + 1) * 512],
                                    start=True, stop=True)
                            nc.scalar.activation(
                                PT[:, rg, :], stp[:], EXP, scale=0.125)

                        scores(0)
                        for kt in range(NKT):
                            if kt + 1 < NKT:
                                scores(kt + 1)
                            rg = kt % RING
                            for qh, pvh in ((0, pva), (1, pvb)):
                                nc.tensor.matmul(
                                    pvh[:], Vn[:, kt, hh, 0:65],
                                    PT[:, rg, qh * 512:(qh + 1) * 512],
                                    start=(kt == 0), stop=(kt == NKT - 1))
                            if kt % 4 == 3 and pend:
                                emit_ctxn_unit(pdx, *pend.pop(0),
                                               cp_eng="dve")
                        for qh, pvh in ((0, pva), (1, pvb)):
                            s2 = slice(q0 + qh * 512, q0 + (qh + 1) * 512)
                            if qh == 0:
                                nc.vector.tensor_copy(
                                    ctxTz[dr:dr + 64, pr, s2], pvh[0:D, :])
                                nc.vector.tensor_copy(
                                    den2[32 * hh:32 * hh + 1, 0:512],
                                    pvh[D:D + 1, :])
                            else:
                                nc.scalar.copy(
                                    ctxTz[dr:dr + 64, pr, s2], pvh[0:D, :])
                                nc.scalar.copy(
                                    den2[32 * hh:32 * hh + 1, 512:1024],
                                    pvh[D:D + 1, :])
                    while pend:
                        emit_ctxn_unit(pdx, *pend.pop(0), cp_eng="dve")
                    dnp_t = pdx.tile([128, 64], F32, tag="t")
                    for qt in range(QW // 128):
                        nc.tensor.matmul(
                            dnp_t[:, qt * 4:(qt + 1) * 4],
                            den2[:, qt * 128:(qt + 1) * 128],
                            identf[:, 0:97:32], start=True, stop=True)
                    nc.vector.reciprocal(
                        denr[:, qw * 8:(qw + 1) * 8, :],
                        dnp_t[:, 0:32].rearrange("p (a b) -> p a b", a=8))
                pend = [(qw, stl, pr) for stl in range(QW // 128)
                        for pr in range(NPAIR)]

            # ---- final tail: dense(qw0), ctxn+dense(qw1) ----
            with (
                tc.tile_pool(name="ptf", bufs=2,
                             space=bass.MemorySpace.PSUM) as ptf,
                tc.tile_pool(name="pso", bufs=3,
                             space=bass.MemorySpace.PSUM) as pso,
            ):
                def emit_dense(uqw, stl):
                    st = uqw * 8 + stl
                    ssl = slice(st * 128, (st + 1) * 128)
                    for nt in range(HID // 512):
                        nsl = slice(nt * 512, (nt + 1) * 512)
                        po = pso.tile([128, 512], F32, tag="dn")
                        nc.tensor.matmul(po[:], ctxn[:, 0, ssl],
                                         wd_sb[:, 0, nsl],
                                         start=True, stop=False)
                        nc.tensor.matmul(po[:], ctxn[:, 1, ssl],
                                         wd_sb[:, 1, nsl],
                                         start=False, stop=True)
                        ob = obp.tile([128, 512], BF16)
                        if (st + nt) % 2 == 0:
                            nc.scalar.copy(ob[:], po[:])
                        else:
                            nc.vector.tensor_copy(ob[:], po[:])
                        oq = (nc.sync, nc.scalar, nc.gpsimd)[
                            (st * 2 + nt) % 3]
                        oq.dma_start(out[ssl, nsl], ob[:])

                for stl in range(QW // 128):
                    emit_dense(0, stl)
                    emit_ctxn_unit(ptf, *pend.pop(0), cp_eng="act")
                    emit_ctxn_unit(ptf, *pend.pop(0), cp_eng="act")
                for stl in range(QW // 128):
                    emit_dense(1, stl)

    nc.compile()
    return nc


_NC_CACHE = None


def get_nc():
    global _NC_CACHE
    if _NC_CACHE is None:
        _NC_CACHE = build_nc()
    return _NC_CACHE


def make_in_maps(hidden_states, w_qkv, b_qkv, w_dense):
    hs = np.asarray(hidden_states, dtype=np.float32)
    w_qkv = np.asarray(w_qkv, dtype=np.float32)
    b_qkv = np.asarray(b_qkv, dtype=np.float32)
    w_dense = np.asarray(w_dense, dtype=np.float32)
    # Reference column order: per-head [q_h | k_h | v_h] blocks of D.
    qcols = np.concatenate(
        [np.arange(h * 3 * D, h * 3 * D + D) for h in range(HEADS)])
    kcols = qcols + D
    hsT_b = [np.ascontiguousarray(hs[b].T).astype(BF) for b in range(B)]
    in_maps = []
    for c in range(NCORES):
        b, g = c // GROUPS, c % GROUPS
        sel = slice(g * CW, (g + 1) * CW)
        in_maps.append({
            "hsT": hsT_b[b],
            "wq": np.ascontiguousarray(w_qkv[:, qcols[sel]]).astype(BF),
            "wk": np.ascontiguousarray(w_qkv[:, kcols[sel]]).astype(BF),
            "wv": np.ascontiguousarray(w_qkv[:, kcols[sel] + D]).astype(BF),
            "bq": np.ascontiguousarray(b_qkv[qcols[sel]].reshape(CW, 1)),
            "bk": np.ascontiguousarray(b_qkv[kcols[sel]].reshape(CW, 1)),
            "wd": np.ascontiguousarray(w_dense[sel, :]).astype(BF),
        })
    return in_maps


def run(hidden_states, w_qkv, b_qkv, w_dense, b_dense, trace=False):
    nc = get_nc()
    in_maps = make_in_maps(hidden_states, w_qkv, b_qkv, w_dense)
    res = run_bass_kernel_spmd(nc, in_maps, core_ids=list(range(NCORES)),
                               trace=trace)
    w_dense = np.asarray(w_dense, dtype=np.float32)
    b_qkv = np.asarray(b_qkv, dtype=np.float32)
    b_v = np.concatenate(
        [b_qkv[h * 3 * D + 2 * D:h * 3 * D + 3 * D] for h in range(HEADS)])
    tail = (b_v @ w_dense + np.asarray(b_dense, dtype=np.float32))
    full = np.zeros((B, S, HID), np.float32)
    for c in range(NCORES):
        full[c // GROUPS] += np.asarray(res.results[c]["out"]).astype(
            np.float32)
    full += tail
    return full.astype(np.float32), res


def kernel(hidden_states, w_qkv, b_qkv, w_dense, b_dense):
    out, _ = run(hidden_states, w_qkv, b_qkv, w_dense, b_dense,
                 trace=bool(os.environ.get("BASS_TRACE")))
    return out
